# revision 1
# baseline (speedup 1.0000x reference)
"""Trainium2 Bass kernel for nn_CroAttention (cosine cross-attention block).

Computation (per (b,t) pair, 64 pairs total):
  qh  = l2norm_heads(q @ Wq.T + bq)          (256, 8, 64)
  k,v = l2norm_heads(kv @ Wkv.T + bkv)       (512, 8, 64) each
  att = softmax(qh @ kh.T / 8)  per head     (cosine scores in [-1/8, 1/8])
  x   = att @ vh  -> merge heads
  out = x @ Wm.T + bm + q

Sharding: data-parallel over the 64 fused (b,t) pairs -> 8 pairs per core.

Device dataflow (everything stays transposed; no PE transposes needed):
  - host feeds qT [c, lq], kvT [c, lk] per pair and pre-transposed weights
  - Q/K projections emit qh^T [(h d), lq], kh^T [(h d), lk] directly
  - V projection emits vh [lk, (h d)] (natural), plus a ones column per head
  - scores^T[k, l] = kh^T.T @ qh^T per head; exp via ACT (no max needed:
    cosine scores are bounded), row sums ride along as the ones-column row
    of the PV matmul; final out^T = WmT.T @ x^T (+bm +qT) is DMA'd out and
    un-transposed on the host.

All matmuls run as float32r (1 cycle/row on TRN2 when N>=256). Walrus
requires every producer of an f32r matmul operand to write rounded f32r,
so those tiles are float32r natively; non-matmul readers bitcast to f32.
"""

import sys

sys.path.insert(0, "/opt/trn_rl_repo")

import numpy as np

import concourse.bass as bass
import concourse.mybir as mybir
import concourse.tile as tile
from concourse import bacc
from concourse.bass_utils import run_bass_kernel_spmd

F32 = mybir.dt.float32
F32R = mybir.dt.float32r
I32 = mybir.dt.int32
AF = mybir.ActivationFunctionType

LN2 = 0.6931471805599453
MU = 0.0450


def _bits_exp_coefs(p):
    """exp(a*bits(s) + b) ~= s**p via the float-bits logarithm."""
    return p * LN2 / (2 ** 23), -p * LN2 * (127 - MU)

C = 512          # channels
H = 8            # heads
D = 64           # head dim
LQ = 256         # query length
LK = 512         # kv length
P = 8            # (b,t) pairs per core
NCORES = 8
NC_CHUNK = 4     # c split into 4 chunks of 128
EPS2 = 1e-24     # eps^2 for max(norm^2, .) ; sqrt(1e-24) = 1e-12 = torch eps


def f32(ap):
    """Read a float32r tile as plain f32 (values are already rounded)."""
    return ap.bitcast(F32)


def _patch_act_tables():
    """Restrict the ACT table-set choice to natural_log_exp_and_others.

    The kernel only uses Identity/Exp/Ln activations, all present in that
    one set. Left to itself the set chooser flip-flops between the exp and
    ln sets (~12 table loads per pair at ~2.7us each)."""
    orig = bacc.get_activation_tables

    def patched(arch):
        tabs = orig(arch)
        name = "natural_log_exp_and_others"
        if name in tabs:
            return {name: tabs[name]}
        return tabs

    bacc.get_activation_tables = patched


def build_program():
    _patch_act_tables()
    nc = bacc.Bacc(
        "TRN2", target_bir_lowering=False, debug=False, enable_asserts=False
    )

    # ---- DRAM I/O (per core). Matmul-feeding tensors are float32r. ----
    qT_d = nc.dram_tensor("qT", [P * C, LQ], F32R, kind="ExternalInput").ap()
    kvT_d = nc.dram_tensor("kvT", [P * C, LK], F32R, kind="ExternalInput").ap()
    wqT_d = nc.dram_tensor("wqT", [C, C], F32R, kind="ExternalInput").ap()
    wkT_d = nc.dram_tensor("wkT", [C, C], F32R, kind="ExternalInput").ap()
    wvT_d = nc.dram_tensor("wvT", [C, C], F32R, kind="ExternalInput").ap()
    # wm8[d, h, o] = Wm[o, h*64+d]: each head's K=64 chunk starts at partition 0
    wm8_d = nc.dram_tensor("wm8", [D, H, C], F32R, kind="ExternalInput").ap()
    bq_d = nc.dram_tensor("bq", [128, NC_CHUNK], F32, kind="ExternalInput").ap()
    bk_d = nc.dram_tensor("bk", [128, NC_CHUNK], F32, kind="ExternalInput").ap()
    bm_d = nc.dram_tensor("bm", [128, NC_CHUNK], F32, kind="ExternalInput").ap()
    bv_d = nc.dram_tensor("bv", [1, C], F32, kind="ExternalInput").ap()
    # ind[h, o] = 1.0 if o // 64 == h else 0  (bcast inv-norm rows -> o lanes)
    ind_d = nc.dram_tensor("ind", [H, C], F32R, kind="ExternalInput").ap()
    # blk[o, h] = ind.T (accumulate per-head sums of squares)
    blk_d = nc.dram_tensor("blk", [C, H], F32R, kind="ExternalInput").ap()
    # colsel[p, h, m] = 1.0 if m == h (route head h's softmax sums to row h)
    colsel_d = nc.dram_tensor("colsel", [128, H, H], F32R, kind="ExternalInput").ap()
    outT_d = nc.dram_tensor("outT", [P * C, LQ], F32, kind="ExternalOutput").ap()

    with tile.TileContext(nc) as tc:
        with (
            tc.tile_pool(name="singles", bufs=1) as singles,
            tc.tile_pool(name="qin", bufs=2) as qin_pool,
            tc.tile_pool(name="kvin", bufs=2) as kvin_pool,
            tc.tile_pool(name="qh", bufs=2) as qh_pool,
            tc.tile_pool(name="kh", bufs=2) as kh_pool,
            tc.tile_pool(name="vh", bufs=2) as vh_pool,
            tc.tile_pool(name="sq", bufs=2) as sq_pool,
            tc.tile_pool(name="inv", bufs=1) as inv_pool,
            tc.tile_pool(name="nv", bufs=2) as nv_pool,
            tc.tile_pool(name="es", bufs=H) as es_pool,
            tc.tile_pool(name="rc", bufs=2) as rc_pool,
            tc.tile_pool(name="xT", bufs=8) as xT_pool,
            tc.tile_pool(name="outs", bufs=2) as out_pool,
            tc.tile_pool(name="ps", bufs=8, space="PSUM") as ps_pool,
        ):
            # ---- persistent tiles ----
            w_sb = {}
            for name, d in (("wq", wqT_d), ("wk", wkT_d), ("wv", wvT_d)):
                t = singles.tile([128, NC_CHUNK, C], F32R, tag=f"w_{name}")
                nc.sync.dma_start(out=t, in_=d.rearrange("(kc p) o -> p kc o", p=128))
                w_sb[name] = t
            wm8_sb = singles.tile([D, H, C], F32R, tag="w_wm")
            nc.sync.dma_start(out=wm8_sb, in_=wm8_d)
            bq_sb = singles.tile([128, NC_CHUNK], F32, tag="bq")
            nc.sync.dma_start(out=bq_sb, in_=bq_d)
            bk_sb = singles.tile([128, NC_CHUNK], F32, tag="bk")
            nc.sync.dma_start(out=bk_sb, in_=bk_d)
            bm_sb = singles.tile([128, NC_CHUNK], F32, tag="bm")
            nc.sync.dma_start(out=bm_sb, in_=bm_d)
            bv_sb = singles.tile([128, C], F32, tag="bv")
            nc.sync.dma_start(out=bv_sb, in_=bv_d.to_broadcast([128, C]))
            ind_sb = singles.tile([H, C], F32R, tag="ind")
            nc.sync.dma_start(out=ind_sb, in_=ind_d)
            blk_sb = singles.tile([128, NC_CHUNK, H], F32R, tag="blk")
            nc.sync.dma_start(out=blk_sb, in_=blk_d.rearrange("(j p) h -> p j h", p=128))
            colsel_sb = singles.tile([128, H, H], F32R, tag="colsel")
            nc.sync.dma_start(out=colsel_sb, in_=colsel_d)
            # bias rows for the bits-log exp seeds (s^-1/2 and s^-1)
            a_h, b_h = _bits_exp_coefs(-0.5)
            a_r, b_r = _bits_exp_coefs(-1.0)
            bh_sb = singles.tile([128, 1], F32, tag="bh")
            nc.vector.memset(bh_sb, b_h)
            br_sb = singles.tile([128, 1], F32, tag="br")
            nc.vector.memset(br_sb, b_r)
            eps2_bits = int(np.float32(EPS2).view(np.int32))

            def rsqrt_rounds(out_r, s_ps, pool, npart, n, tag, p, nr, clamp):
                """out_r (f32r) = s_ps ** p via bits-log exp seed + nr Newton
                steps. s_ps is a PSUM f32 AP; clamp applies max(s, eps^2) in
                the int domain (valid for positive floats). PSUM-touching ops
                run on DVE, pure-SBUF elementwise on the idle GPSIMD."""
                a, b, bias = (a_h, b_h, bh_sb) if p == -0.5 else (a_r, b_r, br_sb)
                bits_f = pool.tile([npart, n], F32, tag=tag + "b")
                if clamp:
                    nc.vector.tensor_scalar(
                        out=bits_f, in0=s_ps.bitcast(I32),
                        scalar1=eps2_bits, scalar2=None,
                        op0=mybir.AluOpType.max,
                    )
                else:
                    nc.vector.tensor_copy(bits_f, s_ps.bitcast(I32))
                y = pool.tile([npart, n], F32, tag=tag + "y")
                nc.scalar.activation(
                    out=y, in_=bits_f, func=AF.Exp, scale=a, bias=bias[0:npart, :]
                )
                for it in range(nr):
                    t = pool.tile([npart, n], F32, tag=tag + "t")
                    if p == -0.5:
                        # t = s*y^2 ; y <- y*(1.5 - 0.5 t)
                        nc.vector.tensor_mul(t, y, y)
                        nc.vector.tensor_mul(t, t, s_ps)
                        nc.vector.tensor_scalar(
                            out=t, in0=t, scalar1=-0.5, scalar2=1.5,
                            op0=mybir.AluOpType.mult, op1=mybir.AluOpType.add,
                        )
                    else:
                        # t = s*y ; y <- y*(2 - t)
                        nc.vector.tensor_mul(t, y, s_ps)
                        nc.vector.tensor_scalar(
                            out=t, in0=t, scalar1=-1.0, scalar2=2.0,
                            op0=mybir.AluOpType.mult, op1=mybir.AluOpType.add,
                        )
                    last = it == nr - 1
                    yn = out_r if last else pool.tile([npart, n], F32, tag=tag + "y")
                    nc.vector.tensor_mul(yn, t, y)
                    y = yn

            for i in range(P):
                # ---- load inputs (transposed): q^T [c, lq], kv^T [c, lk] ----
                q_sb = qin_pool.tile([128, NC_CHUNK, LQ], F32R, tag="qin")
                nc.sync.dma_start(
                    out=q_sb,
                    in_=qT_d[i * C:(i + 1) * C, :].rearrange("(j p) l -> p j l", p=128),
                )
                kv_sb = kvin_pool.tile([128, NC_CHUNK, LK], F32R, tag="kvin")
                nc.sync.dma_start(
                    out=kv_sb,
                    in_=kvT_d[i * C:(i + 1) * C, :].rearrange("(j p) l -> p j l", p=128),
                )

                # ---- Q / K projections -> transposed heads + l2-normalize ----
                # qh^T[(h d), lq] ; kh^T[(h d), lk]
                def proj_t(w, bias_sb, in_sb, n, hname):
                    h_sb = (qh_pool if hname == "q" else kh_pool).tile(
                        [128, NC_CHUNK, n], F32R, tag=hname + "h"
                    )
                    norm_ps = ps_pool.tile([H, n], F32, tag="ps")
                    for j in range(NC_CHUNK):
                        ps = ps_pool.tile([128, n], F32, tag="ps")
                        for kc in range(NC_CHUNK):
                            nc.tensor.matmul(
                                ps,
                                lhsT=w[:, kc, j * 128:(j + 1) * 128],
                                rhs=in_sb[:, kc, :],
                                start=(kc == 0),
                                stop=(kc == NC_CHUNK - 1),
                            )
                        # add bias while copying PSUM -> SBUF (rounds to f32r)
                        nc.vector.tensor_scalar(
                            out=h_sb[:, j, :], in0=ps,
                            scalar1=bias_sb[:, j:j + 1], scalar2=None,
                            op0=mybir.AluOpType.add,
                        )
                        # accumulate per-head squared norms: [H, n]
                        sq = sq_pool.tile([128, n], F32R, tag="sq")
                        nc.vector.tensor_mul(sq, f32(h_sb[:, j, :]), f32(h_sb[:, j, :]))
                        nc.tensor.matmul(
                            norm_ps,
                            lhsT=blk_sb[:, j, :],
                            rhs=sq,
                            start=(j == 0),
                            stop=(j == NC_CHUNK - 1),
                        )
                    # inv = 1/max(sqrt(s), eps) = s^-0.5 with s clamped
                    inv_sb = inv_pool.tile([H, n], F32R, tag="inv" + hname)
                    rsqrt_rounds(
                        inv_sb, norm_ps, inv_pool, H, n, "inv" + hname,
                        p=-0.5, nr=1, clamp=True,
                    )
                    # h *= inv (broadcast head rows over their 64 partitions)
                    for j in range(NC_CHUNK):
                        bc = ps_pool.tile([128, n], F32, tag="ps")
                        nc.tensor.matmul(
                            bc,
                            lhsT=ind_sb[:, j * 128:(j + 1) * 128],
                            rhs=inv_sb,
                            start=True, stop=True,
                        )
                        nc.vector.tensor_mul(h_sb[:, j, :], f32(h_sb[:, j, :]), bc)
                    return h_sb

                qh_sb = proj_t(w_sb["wq"], bq_sb, q_sb, LQ, "q")
                kh_sb = proj_t(w_sb["wk"], bk_sb, kv_sb, LK, "k")

                # ---- V projection (natural layout) + l2-normalize ----
                # vh[lk, h, d] = l2norm(kv @ Wv.T + bv)
                vh_sb = vh_pool.tile([128, NC_CHUNK, H, D], F32R, tag="vh")
                nv_all = nv_pool.tile([128, NC_CHUNK, H], F32, tag="nv")
                for j in range(NC_CHUNK):  # lk chunk
                    ps = ps_pool.tile([128, C], F32, tag="ps")
                    for kc in range(NC_CHUNK):
                        nc.tensor.matmul(
                            ps,
                            lhsT=kv_sb[:, kc, j * 128:(j + 1) * 128],
                            rhs=w_sb["wv"][:, kc, :],
                            start=(kc == 0),
                            stop=(kc == NC_CHUNK - 1),
                        )
                    nc.vector.tensor_add(
                        vh_sb[:, j, :, :],
                        ps.rearrange("p (h d) -> p h d", h=H),
                        bv_sb.rearrange("p (h d) -> p h d", h=H),
                    )
                    sqv = sq_pool.tile([128, C], F32, tag="sqv")
                    sqv3 = sqv.rearrange("p (h d) -> p h d", h=H)
                    nc.vector.tensor_mul(
                        sqv3, f32(vh_sb[:, j, :, :]), f32(vh_sb[:, j, :, :])
                    )
                    nc.vector.reduce_sum(
                        nv_all[:, j, :], sqv3, axis=mybir.AxisListType.X
                    )
                # rsqrt of all 32 norms at once
                nvr = nv_pool.tile([128, NC_CHUNK, H], F32, tag="nvr")
                rsqrt_rounds(
                    nvr.rearrange("p a b -> p (a b)"),
                    nv_all.rearrange("p a b -> p (a b)"),
                    nv_pool, 128, NC_CHUNK * H, "nv",
                    p=-0.5, nr=1, clamp=True,
                )
                for j in range(NC_CHUNK):
                    nc.vector.tensor_mul(
                        vh_sb[:, j, :, :],
                        f32(vh_sb[:, j, :, :]),
                        nvr[:, j, :].broadcast_to([128, H, D]),
                    )

                # ---- attention: pass 1 = scores+exp per head (es in SBUF),
                # all softmax denominators accumulated into one [8, lq] PSUM
                # tile (colsel routes head h's column-sums to row h) ----
                sums_ps = ps_pool.tile([H, LQ], F32, tag="ps")
                es_all = []
                for h in range(H):
                    jh, ph = h // 2, (h % 2) * D
                    es_sb = es_pool.tile([128, NC_CHUNK, LQ], F32R, tag="es")
                    for jkk in range(NC_CHUNK // 2):  # pairs of lk chunks
                        ps_s = ps_pool.tile([128, 2, LQ], F32, tag="ps")
                        for s in range(2):
                            jk = 2 * jkk + s
                            nc.tensor.matmul(
                                ps_s[:, s, :],
                                lhsT=kh_sb[ph:ph + D, jh, jk * 128:(jk + 1) * 128],
                                rhs=qh_sb[ph:ph + D, jh, :],
                                start=True, stop=True,
                            )
                        # att = exp(scores / sqrt(D)); cosine scores are in
                        # [-1, 1] so no max-subtraction is needed
                        nc.scalar.activation(
                            out=es_sb[:, 2 * jkk:2 * jkk + 2, :], in_=ps_s,
                            func=AF.Exp, scale=0.125,
                        )
                        for s in range(2):
                            jk = 2 * jkk + s
                            nc.tensor.matmul(
                                sums_ps,
                                lhsT=colsel_sb[:, h, :],
                                rhs=es_sb[:, jk, :],
                                start=(h == 0 and jk == 0),
                                stop=(h == H - 1 and jk == NC_CHUNK - 1),
                            )
                    es_all.append(es_sb)
                # reciprocals of all 8 denominators: 1/s = s^-1
                rec_r = rc_pool.tile([H, LQ], F32R, tag="rcr")
                rsqrt_rounds(
                    rec_r, sums_ps, rc_pool, H, LQ, "rec", p=-1.0, nr=2, clamp=False
                )
                # ---- pass 2: PV + normalize per head ----
                xT = []  # per-head x^T [64, lq] tiles (c-chunk rows for out proj)
                for h in range(H):
                    es_sb = es_all[h]
                    ps_x = ps_pool.tile([D, LQ], F32, tag="ps")
                    for jk in range(NC_CHUNK):
                        nc.tensor.matmul(
                            ps_x,
                            lhsT=vh_sb[:, jk, h, :],
                            rhs=es_sb[:, jk, :],
                            start=(jk == 0),
                            stop=(jk == NC_CHUNK - 1),
                        )
                    bc = ps_pool.tile([D, LQ], F32, tag="ps")
                    nc.tensor.matmul(
                        bc, lhsT=ind_sb[:, h * D:(h + 1) * D], rhs=rec_r,
                        start=True, stop=True,
                    )
                    # DVE cannot read two PSUM operands; stage bc via ACT
                    bc_sb = rc_pool.tile([D, LQ], F32, tag="bcsb")
                    nc.scalar.activation(out=bc_sb, in_=bc, func=AF.Identity)
                    xt = xT_pool.tile([D, LQ], F32R, tag="xT")
                    nc.vector.tensor_mul(xt, ps_x, bc_sb)
                    xT.append(xt)

                # ---- output projection + bias + residual (transposed) ----
                out_sb = out_pool.tile([128, NC_CHUNK, LQ], F32, tag="outs")
                for jo in range(NC_CHUNK):
                    ps_o = ps_pool.tile([128, LQ], F32, tag="ps")
                    for h in range(H):  # K chunks of 64 (one per head)
                        nc.tensor.matmul(
                            ps_o,
                            lhsT=wm8_sb[:, h, jo * 128:(jo + 1) * 128],
                            rhs=xT[h],
                            start=(h == 0),
                            stop=(h == H - 1),
                        )
                    # out = ps_o + bm + qT  (fused bias + residual)
                    nc.vector.scalar_tensor_tensor(
                        out=out_sb[:, jo, :],
                        in0=ps_o,
                        scalar=bm_sb[:, jo:jo + 1],
                        in1=f32(q_sb[:, jo, :]),
                        op0=mybir.AluOpType.add,
                        op1=mybir.AluOpType.add,
                    )
                nc.sync.dma_start(
                    out=outT_d[i * C:(i + 1) * C, :].rearrange(
                        "(j p) l -> p j l", p=128
                    ),
                    in_=out_sb,
                )

    nc.compile()
    return nc


_NC_CACHE = None


def _get_program():
    global _NC_CACHE
    if _NC_CACHE is None:
        _NC_CACHE = build_program()
    return _NC_CACHE


def prep_in_maps(q, kv, Wq, bq, Wkv, bkv, Wm, bm):
    q = np.ascontiguousarray(np.asarray(q, dtype=np.float32))
    kv = np.ascontiguousarray(np.asarray(kv, dtype=np.float32))
    b, t, lq, c = q.shape
    lk = kv.shape[2]
    npairs = b * t
    per_core = npairs // NCORES

    # host-side transposes / weight prep (not on the device critical path)
    qT = np.ascontiguousarray(
        q.reshape(npairs, lq, c).transpose(0, 2, 1)
    )  # [64, c, lq]
    kvT = np.ascontiguousarray(
        kv.reshape(npairs, lk, c).transpose(0, 2, 1)
    )  # [64, c, lk]
    wqT = np.ascontiguousarray(np.asarray(Wq, np.float32).T)           # [c, c]
    wkT = np.ascontiguousarray(np.asarray(Wkv[:C], np.float32).T)      # [c, c]
    wvT = np.ascontiguousarray(np.asarray(Wkv[C:], np.float32).T)      # [c, c]
    # wm8[d, h, o] = Wm[o, h*64+d]
    wm8 = np.ascontiguousarray(
        np.asarray(Wm, np.float32).T.reshape(H, D, C).transpose(1, 0, 2)
    )
    bq_t = np.ascontiguousarray(np.asarray(bq, np.float32).reshape(NC_CHUNK, 128).T)
    bk_t = np.ascontiguousarray(
        np.asarray(bkv[:C], np.float32).reshape(NC_CHUNK, 128).T
    )
    bv_t = np.ascontiguousarray(np.asarray(bkv[C:], np.float32).reshape(1, C))
    bm_t = np.ascontiguousarray(np.asarray(bm, np.float32).reshape(NC_CHUNK, 128).T)
    ind = np.zeros((H, C), np.float32)
    for h in range(H):
        ind[h, h * D:(h + 1) * D] = 1.0
    blk = np.ascontiguousarray(ind.T)
    colsel = np.zeros((128, H, H), np.float32)
    for h in range(H):
        colsel[:, h, h] = 1.0

    in_maps = []
    for core in range(NCORES):
        s = core * per_core
        e = s + per_core
        in_maps.append({
            "qT": qT[s:e].reshape(per_core * C, lq),
            "kvT": kvT[s:e].reshape(per_core * C, lk),
            "wqT": wqT, "wkT": wkT, "wvT": wvT, "wm8": wm8,
            "bq": bq_t, "bk": bk_t, "bv": bv_t, "bm": bm_t,
            "ind": ind, "blk": blk, "colsel": colsel,
        })
    return in_maps, (b, t, lq, c, per_core)


def kernel(q, kv, Wq, bq, Wkv, bkv, Wm, bm):
    in_maps, (b, t, lq, c, per_core) = prep_in_maps(q, kv, Wq, bq, Wkv, bkv, Wm, bm)
    nc = _get_program()
    res = run_bass_kernel_spmd(nc, in_maps, core_ids=list(range(NCORES)))
    outT = np.concatenate(
        [res.results[core]["outT"].reshape(per_core, C, lq) for core in range(NCORES)],
        axis=0,
    )  # [64, c, lq]
    out = outT.transpose(0, 2, 1).reshape(b, t, lq, c)
    return np.ascontiguousarray(out)



# revision 6
# speedup vs baseline: 1.6114x; 1.6114x over previous
"""Trainium2 Bass kernel for nn_CroAttention (cosine cross-attention block).

Computation (per (b,t) pair, 64 pairs total):
  qh  = l2norm_heads(q @ Wq.T + bq)          (256, 8, 64)
  k,v = l2norm_heads(kv @ Wkv.T + bkv)       (512, 8, 64) each
  att = softmax(qh @ kh.T / 8)  per head     (cosine scores in [-1/8, 1/8])
  x   = att @ vh  -> merge heads
  out = x @ Wm.T + bm + q

Sharding: data-parallel over the 64 fused (b,t) pairs -> 8 pairs per core.

Device dataflow (everything stays transposed; no PE transposes needed):
  - host feeds qT [c, lq], kvT [c, lk] per pair and pre-transposed weights
  - Q/K projections emit qh^T [(h d), lq], kh^T [(h d), lk] directly
  - V projection emits vh [lk, (h d)] (natural)
  - scores^T[k, l] = kh^T.T @ qh^T per head; exp via ACT (no max needed:
    cosine scores are bounded); softmax denominators via colsel matmuls
  - final out^T = WmT.T @ x^T (+bm +qT) is DMA'd out, un-transposed on host

Scheduling: engines execute their queues IN ORDER, so emission order is
chosen to keep PE busy: all projection matmuls for a pair are emitted
before any instruction that depends on the (serial) inverse-norm chains,
and the output projection of pair i-1 is emitted in the middle of pair
i's projection phase so PE has work while pair i's chains settle.

Head pairing: PV results for heads (2m, 2m+1) land in one [128, lq] PSUM
tile (partitions 0-63 / 64-127), so the x^T tiles give the output
projection K=128 per matmul (16 matmuls instead of 32) and one DVE
normalize mul per pair of heads.
"""

import sys

sys.path.insert(0, "/opt/trn_rl_repo")

import numpy as np

import concourse.bass as bass
import concourse.mybir as mybir
import concourse.tile as tile
from concourse import bacc
from concourse.bass_utils import run_bass_kernel_spmd

F32 = mybir.dt.float32
F32R = mybir.dt.float32r
BF16 = mybir.dt.bfloat16
I32 = mybir.dt.int32
AF = mybir.ActivationFunctionType

LN2 = 0.6931471805599453
MU = 0.0450


def _bits_exp_coefs(p):
    """exp(a*bits(s) + b) ~= s**p via the float-bits logarithm."""
    return p * LN2 / (2 ** 23), -p * LN2 * (127 - MU)

C = 512          # channels
H = 8            # heads
D = 64           # head dim
LQ = 256         # query length
LK = 512         # kv length
P = 8            # (b,t) pairs per core
NCORES = 8
NC_CHUNK = 4     # c split into 4 chunks of 128
EPS2 = 1e-24     # eps^2 for max(norm^2, .) ; sqrt(1e-24) = 1e-12 = torch eps


def f32(ap):
    """Read a float32r tile as plain f32 (values are already rounded)."""
    return ap.bitcast(F32)


def _patch_act_tables():
    """Restrict the ACT table-set choice to natural_log_exp_and_others.

    The kernel only uses Identity/Exp/Ln activations, all present in that
    one set. Left to itself the set chooser flip-flops between the exp and
    ln sets (~12 table loads per pair at ~2.7us each)."""
    orig = bacc.get_activation_tables

    def patched(arch):
        tabs = orig(arch)
        name = "natural_log_exp_and_others"
        if name in tabs:
            return {name: tabs[name]}
        return tabs

    bacc.get_activation_tables = patched


def build_program():
    _patch_act_tables()
    nc = bacc.Bacc(
        "TRN2", target_bir_lowering=False, debug=False, enable_asserts=False
    )

    # ---- DRAM I/O (per core). Matmul-feeding tensors are float32r. ----
    qT_d = nc.dram_tensor("qT", [P * C, LQ], BF16, kind="ExternalInput").ap()
    kvT_d = nc.dram_tensor("kvT", [P * C, LK], BF16, kind="ExternalInput").ap()
    wqT_d = nc.dram_tensor("wqT", [C, C], BF16, kind="ExternalInput").ap()
    wkT_d = nc.dram_tensor("wkT", [C, C], BF16, kind="ExternalInput").ap()
    wvT_d = nc.dram_tensor("wvT", [C, C], BF16, kind="ExternalInput").ap()
    # wmp[p, m, o] = Wm[o, (2m + p//64)*64 + p%64]: K chunk for head pair m
    wmp_d = nc.dram_tensor("wmp", [128, NC_CHUNK, C], BF16, kind="ExternalInput").ap()
    bq_d = nc.dram_tensor("bq", [128, NC_CHUNK], F32, kind="ExternalInput").ap()
    bk_d = nc.dram_tensor("bk", [128, NC_CHUNK], F32, kind="ExternalInput").ap()
    bm_d = nc.dram_tensor("bm", [128, NC_CHUNK], F32, kind="ExternalInput").ap()
    bv_d = nc.dram_tensor("bv", [1, C], F32, kind="ExternalInput").ap()
    # ind[h, o] = 1.0 if o // 64 == h else 0  (bcast inv-norm rows -> o lanes)
    ind_d = nc.dram_tensor("ind", [H, C], BF16, kind="ExternalInput").ap()
    # blk[o, h] = ind.T (accumulate per-head sums of squares)
    blk_d = nc.dram_tensor("blk", [C, H], BF16, kind="ExternalInput").ap()
    # colsel[p, h, m] = 1.0 if m == h (route head h's softmax sums to row h)
    colsel_d = nc.dram_tensor("colsel", [128, H, H], BF16, kind="ExternalInput").ap()
    # indp[k, m, p] = 1.0 if k == 2m + p//64 (bcast rec rows onto head pair m)
    indp_d = nc.dram_tensor("indp", [H, NC_CHUNK, 128], BF16, kind="ExternalInput").ap()
    outT_d = nc.dram_tensor("outT", [P * C, LQ], F32, kind="ExternalOutput").ap()

    with tile.TileContext(nc) as tc:
        with (
            tc.tile_pool(name="singles", bufs=1) as singles,
            tc.tile_pool(name="qin", bufs=2) as qin_pool,
            tc.tile_pool(name="kvin", bufs=2) as kvin_pool,
            tc.tile_pool(name="qh", bufs=2) as qh_pool,
            tc.tile_pool(name="kh", bufs=2) as kh_pool,
            tc.tile_pool(name="vh", bufs=2) as vh_pool,
            tc.tile_pool(name="sq", bufs=2) as sq_pool,
            tc.tile_pool(name="inv", bufs=2) as inv_pool,
            tc.tile_pool(name="nv", bufs=2) as nv_pool,
            tc.tile_pool(name="es", bufs=H) as es_pool,
            tc.tile_pool(name="rc", bufs=2) as rc_pool,
            tc.tile_pool(name="xT", bufs=8) as xT_pool,
            tc.tile_pool(name="outs", bufs=2) as out_pool,
            tc.tile_pool(name="ps", bufs=8, space="PSUM") as ps_pool,
        ):
            # ---- persistent tiles ----
            w_sb = {}
            for name, d in (("wq", wqT_d), ("wk", wkT_d), ("wv", wvT_d)):
                t = singles.tile([128, NC_CHUNK, C], BF16, tag=f"w_{name}")
                nc.sync.dma_start(out=t, in_=d.rearrange("(kc p) o -> p kc o", p=128))
                w_sb[name] = t
            wmp_sb = singles.tile([128, NC_CHUNK, C], BF16, tag="w_wm")
            nc.sync.dma_start(out=wmp_sb, in_=wmp_d)
            bq_sb = singles.tile([128, NC_CHUNK], F32, tag="bq")
            nc.sync.dma_start(out=bq_sb, in_=bq_d)
            bk_sb = singles.tile([128, NC_CHUNK], F32, tag="bk")
            nc.sync.dma_start(out=bk_sb, in_=bk_d)
            bm_sb = singles.tile([128, NC_CHUNK], F32, tag="bm")
            nc.sync.dma_start(out=bm_sb, in_=bm_d)
            bv_sb = singles.tile([128, C], F32, tag="bv")
            nc.sync.dma_start(out=bv_sb, in_=bv_d.to_broadcast([128, C]))
            ind_sb = singles.tile([H, C], BF16, tag="ind")
            nc.sync.dma_start(out=ind_sb, in_=ind_d)
            blk_sb = singles.tile([128, NC_CHUNK, H], BF16, tag="blk")
            nc.sync.dma_start(out=blk_sb, in_=blk_d.rearrange("(j p) h -> p j h", p=128))
            colsel_sb = singles.tile([128, H, H], BF16, tag="colsel")
            nc.sync.dma_start(out=colsel_sb, in_=colsel_d)
            indp_sb = singles.tile([H, NC_CHUNK, 128], BF16, tag="indp")
            nc.sync.dma_start(out=indp_sb, in_=indp_d)
            # bias rows for the bits-log exp seeds (s^-1/2 and s^-1)
            a_h, b_h = _bits_exp_coefs(-0.5)
            a_r, b_r = _bits_exp_coefs(-1.0)
            bh_sb = singles.tile([128, 1], F32, tag="bh")
            nc.vector.memset(bh_sb, b_h)
            br_sb = singles.tile([128, 1], F32, tag="br")
            nc.vector.memset(br_sb, b_r)
            eps2_bits = int(np.float32(EPS2).view(np.int32))

            def rsqrt_rounds(out_r, s_ps, pool, npart, n, tag, p, nr, clamp):
                """out_r (f32r) = s_ps ** p via bits-log exp seed + nr Newton
                steps. s_ps is a PSUM f32 AP; clamp applies max(s, eps^2) in
                the int domain (valid for positive floats)."""
                a, b, bias = (a_h, b_h, bh_sb) if p == -0.5 else (a_r, b_r, br_sb)
                bits_f = pool.tile([npart, n], F32, tag=tag + "b")
                if clamp:
                    nc.vector.tensor_scalar(
                        out=bits_f, in0=s_ps.bitcast(I32),
                        scalar1=eps2_bits, scalar2=None,
                        op0=mybir.AluOpType.max,
                    )
                else:
                    nc.vector.tensor_copy(bits_f, s_ps.bitcast(I32))
                y = pool.tile([npart, n], F32, tag=tag + "y")
                nc.scalar.activation(
                    out=y, in_=bits_f, func=AF.Exp, scale=a, bias=bias[0:npart, :]
                )
                for it in range(nr):
                    t = pool.tile([npart, n], F32, tag=tag + "t")
                    if p == -0.5:
                        # t = s*y^2 ; y <- y*(1.5 - 0.5 t)
                        nc.vector.tensor_mul(t, y, y)
                        nc.vector.tensor_mul(t, t, s_ps)
                        nc.vector.tensor_scalar(
                            out=t, in0=t, scalar1=-0.5, scalar2=1.5,
                            op0=mybir.AluOpType.mult, op1=mybir.AluOpType.add,
                        )
                    else:
                        # t = s*y ; y <- y*(2 - t)
                        nc.vector.tensor_mul(t, y, s_ps)
                        nc.vector.tensor_scalar(
                            out=t, in0=t, scalar1=-1.0, scalar2=2.0,
                            op0=mybir.AluOpType.mult, op1=mybir.AluOpType.add,
                        )
                    last = it == nr - 1
                    yn = out_r if last else pool.tile([npart, n], F32, tag=tag + "y")
                    nc.vector.tensor_mul(yn, t, y)
                    y = yn
                if nr == 0:
                    nc.vector.tensor_copy(out_r, y)

            def proj_mm(w, bias_sb, in_sb, n, hname):
                """Projection matmuls + bias + squared-norm accumulation.
                Emits all 16 projection matmuls first so the DVE bias/sq ops
                overlap later matmuls; norm matmuls go last. Returns the
                (un-normalized) head tile and the [H, n] norm PSUM tile."""
                h_sb = (qh_pool if hname == "q" else kh_pool).tile(
                    [128, NC_CHUNK, n], BF16, tag=hname + "h"
                )
                ps_l = []
                for j in range(NC_CHUNK):
                    ps = ps_pool.tile([128, n], F32, tag="ps")
                    for kc in range(NC_CHUNK):
                        nc.tensor.matmul(
                            ps,
                            lhsT=w[:, kc, j * 128:(j + 1) * 128],
                            rhs=in_sb[:, kc, :],
                            start=(kc == 0),
                            stop=(kc == NC_CHUNK - 1),
                        )
                    ps_l.append(ps)
                sq_l = []
                for j in range(NC_CHUNK):
                    # add bias while copying PSUM -> SBUF (rounds to f32r)
                    nc.vector.tensor_scalar(
                        out=h_sb[:, j, :], in0=ps_l[j],
                        scalar1=bias_sb[:, j:j + 1], scalar2=None,
                        op0=mybir.AluOpType.add,
                    )
                    sq = sq_pool.tile([128, n], BF16, tag="sq" + hname)
                    nc.vector.tensor_mul(sq, h_sb[:, j, :], h_sb[:, j, :])
                    sq_l.append(sq)
                norm_ps = ps_pool.tile([H, n], F32, tag="ps")
                for j in range(NC_CHUNK):
                    nc.tensor.matmul(
                        norm_ps,
                        lhsT=blk_sb[:, j, :],
                        rhs=sq_l[j],
                        start=(j == 0),
                        stop=(j == NC_CHUNK - 1),
                    )
                return h_sb, norm_ps

            def apply_norm(h_sb, inv_sb, n):
                """h *= inv (broadcast head rows over their 64 partitions)."""
                for j in range(NC_CHUNK):
                    bc = ps_pool.tile([128, n], F32, tag="ps")
                    nc.tensor.matmul(
                        bc,
                        lhsT=ind_sb[:, j * 128:(j + 1) * 128],
                        rhs=inv_sb,
                        start=True, stop=True,
                    )
                    nc.vector.tensor_mul(h_sb[:, j, :], h_sb[:, j, :], bc)

            def emit_outproj(xT, q_sb, i):
                # ---- output projection + bias + residual (transposed) ----
                out_sb = out_pool.tile([128, NC_CHUNK, LQ], F32, tag="outs")
                for jo in range(NC_CHUNK):
                    ps_o = ps_pool.tile([128, LQ], F32, tag="ps")
                    for m in range(NC_CHUNK):  # K chunks of 128 (head pair)
                        nc.tensor.matmul(
                            ps_o,
                            lhsT=wmp_sb[:, m, jo * 128:(jo + 1) * 128],
                            rhs=xT[m],
                            start=(m == 0),
                            stop=(m == NC_CHUNK - 1),
                        )
                    # out = ps_o + bm + qT  (fused bias + residual)
                    nc.vector.scalar_tensor_tensor(
                        out=out_sb[:, jo, :],
                        in0=ps_o,
                        scalar=bm_sb[:, jo:jo + 1],
                        in1=q_sb[:, jo, :],
                        op0=mybir.AluOpType.add,
                        op1=mybir.AluOpType.add,
                    )
                nc.sync.dma_start(
                    out=outT_d[i * C:(i + 1) * C, :].rearrange(
                        "(j p) l -> p j l", p=128
                    ),
                    in_=out_sb,
                )

            prev = None
            for i in range(P):
                # ---- load inputs (transposed): q^T [c, lq], kv^T [c, lk] ----
                q_sb = qin_pool.tile([128, NC_CHUNK, LQ], BF16, tag="qin")
                nc.sync.dma_start(
                    out=q_sb,
                    in_=qT_d[i * C:(i + 1) * C, :].rearrange("(j p) l -> p j l", p=128),
                )
                kv_sb = kvin_pool.tile([128, NC_CHUNK, LK], BF16, tag="kvin")
                nc.sync.dma_start(
                    out=kv_sb,
                    in_=kvT_d[i * C:(i + 1) * C, :].rearrange("(j p) l -> p j l", p=128),
                )

                # ---- Q / K / V projection matmul blocks + inv-norm chains ----
                qh_sb, normq_ps = proj_mm(w_sb["wq"], bq_sb, q_sb, LQ, "q")
                invq_sb = inv_pool.tile([H, LQ], BF16, tag="invq")
                rsqrt_rounds(invq_sb, normq_ps, inv_pool, H, LQ, "invq",
                             p=-0.5, nr=1, clamp=True)

                kh_sb, normk_ps = proj_mm(w_sb["wk"], bk_sb, kv_sb, LK, "k")
                invk_sb = inv_pool.tile([H, LK], BF16, tag="invk")
                rsqrt_rounds(invk_sb, normk_ps, inv_pool, H, LK, "invk",
                             p=-0.5, nr=1, clamp=True)

                # V projection (natural layout): vh[lk, h, d] = kv @ Wv.T + bv
                vh_sb = vh_pool.tile([128, NC_CHUNK, H, D], BF16, tag="vh")
                nv_all = nv_pool.tile([128, NC_CHUNK, H], F32, tag="nv")
                ps_v = []
                for j in range(NC_CHUNK):  # lk chunk
                    ps = ps_pool.tile([128, C], F32, tag="ps")
                    for kc in range(NC_CHUNK):
                        nc.tensor.matmul(
                            ps,
                            lhsT=kv_sb[:, kc, j * 128:(j + 1) * 128],
                            rhs=w_sb["wv"][:, kc, :],
                            start=(kc == 0),
                            stop=(kc == NC_CHUNK - 1),
                        )
                    ps_v.append(ps)
                for j in range(NC_CHUNK):
                    nc.vector.tensor_add(
                        vh_sb[:, j, :, :],
                        ps_v[j].rearrange("p (h d) -> p h d", h=H),
                        bv_sb.rearrange("p (h d) -> p h d", h=H),
                    )
                    sqv = sq_pool.tile([128, C], BF16, tag="sqv")
                    sqv3 = sqv.rearrange("p (h d) -> p h d", h=H)
                    nc.vector.tensor_mul(
                        sqv3, vh_sb[:, j, :, :], vh_sb[:, j, :, :]
                    )
                    nc.vector.reduce_sum(
                        nv_all[:, j, :], sqv3, axis=mybir.AxisListType.X
                    )
                # rsqrt of all 32 v norms at once
                nvr = nv_pool.tile([128, NC_CHUNK, H], BF16, tag="nvr")
                rsqrt_rounds(
                    nvr.rearrange("p a b -> p (a b)"),
                    nv_all.rearrange("p a b -> p (a b)"),
                    nv_pool, 128, NC_CHUNK * H, "nv",
                    p=-0.5, nr=1, clamp=True,
                )

                # ---- output projection of the PREVIOUS pair: fills PE while
                # this pair's inverse-norm chains run on DVE/ACT ----
                if prev is not None:
                    emit_outproj(*prev)
                    prev = None

                # ---- apply the inverse norms ----
                apply_norm(qh_sb, invq_sb, LQ)
                apply_norm(kh_sb, invk_sb, LK)
                for j in range(NC_CHUNK):
                    nc.vector.tensor_mul(
                        vh_sb[:, j, :, :],
                        vh_sb[:, j, :, :],
                        nvr[:, j, :].broadcast_to([128, H, D]),
                    )

                # ---- attention pass 1: scores + exp per head (es in SBUF) ----
                es_all = []
                for h in range(H):
                    jh, ph = h // 2, (h % 2) * D
                    es_sb = es_pool.tile([128, NC_CHUNK, LQ], BF16, tag="es")
                    for jkk in range(NC_CHUNK // 2):  # pairs of lk chunks
                        ps_s = ps_pool.tile([128, 2, LQ], F32, tag="ps")
                        for s in range(2):
                            jk = 2 * jkk + s
                            nc.tensor.matmul(
                                ps_s[:, s, :],
                                lhsT=kh_sb[ph:ph + D, jh, jk * 128:(jk + 1) * 128],
                                rhs=qh_sb[ph:ph + D, jh, :],
                                start=True, stop=True,
                            )
                        # att = exp(scores / sqrt(D)); cosine scores are in
                        # [-1, 1] so no max-subtraction is needed
                        nc.scalar.activation(
                            out=es_sb[:, 2 * jkk:2 * jkk + 2, :], in_=ps_s,
                            func=AF.Exp, scale=0.125,
                        )
                    es_all.append(es_sb)
                # all softmax denominators -> one [8, lq] PSUM tile (colsel
                # routes head h's column-sums to row h); emitted after ALL
                # score matmuls so PE never waits on a just-issued exp
                sums_ps = ps_pool.tile([H, LQ], F32, tag="ps")
                for h in range(H):
                    for jk in range(NC_CHUNK):
                        nc.tensor.matmul(
                            sums_ps,
                            lhsT=colsel_sb[:, h, :],
                            rhs=es_all[h][:, jk, :],
                            start=(h == 0 and jk == 0),
                            stop=(h == H - 1 and jk == NC_CHUNK - 1),
                        )
                # reciprocals of all 8 denominators: 1/s = s^-1
                rec_r = rc_pool.tile([H, LQ], BF16, tag="rcr")
                rsqrt_rounds(
                    rec_r, sums_ps, rc_pool, H, LQ, "rec", p=-1.0, nr=1, clamp=False
                )

                # ---- pass 2: PV + normalize, head pairs (2m, 2m+1) share a
                # [128, lq] PSUM tile -> x^T pair tiles with K=128 rows ----
                xT = []
                for m in range(NC_CHUNK):
                    ps_x = ps_pool.tile([128, LQ], F32, tag="ps")
                    for s in range(2):
                        h = 2 * m + s
                        for jk in range(NC_CHUNK):
                            nc.tensor.matmul(
                                ps_x[s * D:(s + 1) * D, :],
                                lhsT=vh_sb[:, jk, h, :],
                                rhs=es_all[h][:, jk, :],
                                start=(jk == 0),
                                stop=(jk == NC_CHUNK - 1),
                            )
                    bc = ps_pool.tile([128, LQ], F32, tag="ps")
                    nc.tensor.matmul(
                        bc, lhsT=indp_sb[:, m, :], rhs=rec_r,
                        start=True, stop=True,
                    )
                    # DVE cannot read two PSUM operands; stage bc via ACT
                    bc_sb = rc_pool.tile([128, LQ], F32, tag="bcsb")
                    nc.scalar.activation(out=bc_sb, in_=bc, func=AF.Identity)
                    xt = xT_pool.tile([128, LQ], BF16, tag="xT")
                    nc.vector.tensor_mul(xt, ps_x, bc_sb)
                    xT.append(xt)

                prev = (xT, q_sb, i)

            emit_outproj(*prev)

    nc.compile()
    return nc


_NC_CACHE = None


def _get_program():
    global _NC_CACHE
    if _NC_CACHE is None:
        _NC_CACHE = build_program()
    return _NC_CACHE


def prep_in_maps(q, kv, Wq, bq, Wkv, bkv, Wm, bm):
    import ml_dtypes

    bf16 = ml_dtypes.bfloat16
    q = np.ascontiguousarray(np.asarray(q, dtype=np.float32))
    kv = np.ascontiguousarray(np.asarray(kv, dtype=np.float32))
    b, t, lq, c = q.shape
    lk = kv.shape[2]
    npairs = b * t
    per_core = npairs // NCORES

    # host-side transposes / weight prep (not on the device critical path)
    qT = np.ascontiguousarray(
        q.reshape(npairs, lq, c).transpose(0, 2, 1).astype(bf16)
    )  # [64, c, lq]
    kvT = np.ascontiguousarray(
        kv.reshape(npairs, lk, c).transpose(0, 2, 1).astype(bf16)
    )  # [64, c, lk]
    wqT = np.ascontiguousarray(np.asarray(Wq, np.float32).T.astype(bf16))
    wkT = np.ascontiguousarray(np.asarray(Wkv[:C], np.float32).T.astype(bf16))
    wvT = np.ascontiguousarray(np.asarray(Wkv[C:], np.float32).T.astype(bf16))
    # wmp[p, m, o] = Wm[o, (2m + p//64)*64 + p%64]
    wmp = np.ascontiguousarray(
        np.asarray(Wm, np.float32).T.reshape(NC_CHUNK, 128, C)
        .transpose(1, 0, 2).astype(bf16)
    )
    bq_t = np.ascontiguousarray(np.asarray(bq, np.float32).reshape(NC_CHUNK, 128).T)
    bk_t = np.ascontiguousarray(
        np.asarray(bkv[:C], np.float32).reshape(NC_CHUNK, 128).T
    )
    bv_t = np.ascontiguousarray(np.asarray(bkv[C:], np.float32).reshape(1, C))
    bm_t = np.ascontiguousarray(np.asarray(bm, np.float32).reshape(NC_CHUNK, 128).T)
    ind = np.zeros((H, C), bf16)
    for h in range(H):
        ind[h, h * D:(h + 1) * D] = 1.0
    blk = np.ascontiguousarray(np.zeros((C, H), bf16))
    for h in range(H):
        blk[h * D:(h + 1) * D, h] = 1.0
    colsel = np.zeros((128, H, H), bf16)
    for h in range(H):
        colsel[:, h, h] = 1.0
    indp = np.zeros((H, NC_CHUNK, 128), bf16)
    for m in range(NC_CHUNK):
        indp[2 * m, m, 0:64] = 1.0
        indp[2 * m + 1, m, 64:128] = 1.0

    in_maps = []
    for core in range(NCORES):
        s = core * per_core
        e = s + per_core
        in_maps.append({
            "qT": qT[s:e].reshape(per_core * C, lq),
            "kvT": kvT[s:e].reshape(per_core * C, lk),
            "wqT": wqT, "wkT": wkT, "wvT": wvT, "wmp": wmp,
            "bq": bq_t, "bk": bk_t, "bv": bv_t, "bm": bm_t,
            "ind": ind, "blk": blk, "colsel": colsel, "indp": indp,
        })
    return in_maps, (b, t, lq, c, per_core)


def kernel(q, kv, Wq, bq, Wkv, bkv, Wm, bm):
    in_maps, (b, t, lq, c, per_core) = prep_in_maps(q, kv, Wq, bq, Wkv, bkv, Wm, bm)
    nc = _get_program()
    res = run_bass_kernel_spmd(nc, in_maps, core_ids=list(range(NCORES)))
    outT = np.concatenate(
        [res.results[core]["outT"].reshape(per_core, C, lq) for core in range(NCORES)],
        axis=0,
    )  # [64, c, lq]
    out = outT.transpose(0, 2, 1).reshape(b, t, lq, c)
    return np.ascontiguousarray(out)


# revision 14
# speedup vs baseline: 1.7537x; 1.0883x over previous
"""Trainium2 Bass kernel for nn_CroAttention (cosine cross-attention block).

Computation (per (b,t) pair, 64 pairs total):
  qh  = l2norm_heads(q @ Wq.T + bq)          (256, 8, 64)
  k,v = l2norm_heads(kv @ Wkv.T + bkv)       (512, 8, 64) each
  att = softmax(qh @ kh.T / 8)  per head     (cosine scores in [-1/8, 1/8])
  x   = att @ vh  -> merge heads
  out = x @ Wm.T + bm + q

Sharding: data-parallel over the 64 fused (b,t) pairs -> 8 pairs per core.

Device dataflow (everything stays transposed; no PE transposes needed):
  - host feeds qT [c, lq], kvT [c, lk] per pair and pre-transposed weights
  - Q/K projections emit qh^T [(h d), lq], kh^T [(h d), lk] directly
  - V projection emits vh [lk, (h d)] (natural)
  - scores^T[k, l] = kh^T.T @ qh^T per head; exp via ACT (no max needed:
    cosine scores are bounded); softmax denominators via colsel matmuls
  - final out^T = WmT.T @ x^T (+bm +qT) is DMA'd out, un-transposed on host

Scheduling: engines execute their queues IN ORDER, so emission order is
chosen to keep PE busy: all projection matmuls for a pair are emitted
before any instruction that depends on the (serial) inverse-norm chains,
and the output projection of pair i-1 is emitted in the middle of pair
i's projection phase so PE has work while pair i's chains settle.

Head pairing: PV results for heads (2m, 2m+1) land in one [128, lq] PSUM
tile (partitions 0-63 / 64-127), so the x^T tiles give the output
projection K=128 per matmul (16 matmuls instead of 32) and one DVE
normalize mul per pair of heads.
"""

import sys

sys.path.insert(0, "/opt/trn_rl_repo")

import numpy as np

import concourse.bass as bass
import concourse.mybir as mybir
import concourse.tile as tile
from concourse import bacc
from concourse.bass_utils import run_bass_kernel_spmd

F32 = mybir.dt.float32
F32R = mybir.dt.float32r
BF16 = mybir.dt.bfloat16
I32 = mybir.dt.int32
AF = mybir.ActivationFunctionType

LN2 = 0.6931471805599453
MU = 0.0450


def _bits_exp_coefs(p):
    """exp(a*bits(s) + b) ~= s**p via the float-bits logarithm."""
    return p * LN2 / (2 ** 23), -p * LN2 * (127 - MU)

C = 512          # channels
H = 8            # heads
D = 64           # head dim
LQ = 256         # query length
LK = 512         # kv length
P = 8            # (b,t) pairs per core
NCORES = 8
NC_CHUNK = 4     # c split into 4 chunks of 128
EPS2 = 1e-24     # eps^2 for max(norm^2, .) ; sqrt(1e-24) = 1e-12 = torch eps


def f32(ap):
    """Read a float32r tile as plain f32 (values are already rounded)."""
    return ap.bitcast(F32)


def _patch_act_tables():
    """Restrict the ACT table-set choice to natural_log_exp_and_others.

    The kernel only uses Identity/Exp/Ln activations, all present in that
    one set. Left to itself the set chooser flip-flops between the exp and
    ln sets (~12 table loads per pair at ~2.7us each)."""
    orig = bacc.get_activation_tables

    def patched(arch):
        tabs = orig(arch)
        name = "natural_log_exp_and_others"
        if name in tabs:
            return {name: tabs[name]}
        return tabs

    bacc.get_activation_tables = patched


def build_program():
    _patch_act_tables()
    nc = bacc.Bacc(
        "TRN2", target_bir_lowering=False, debug=False, enable_asserts=False
    )

    # ---- DRAM I/O (per core). Matmul-feeding tensors are float32r. ----
    qT_d = nc.dram_tensor("qT", [P * C, LQ], BF16, kind="ExternalInput").ap()
    kvT_d = nc.dram_tensor("kvT", [P * C, LK], BF16, kind="ExternalInput").ap()
    wqT_d = nc.dram_tensor("wqT", [C, C], BF16, kind="ExternalInput").ap()
    wkT_d = nc.dram_tensor("wkT", [C, C], BF16, kind="ExternalInput").ap()
    wvT_d = nc.dram_tensor("wvT", [C, C], BF16, kind="ExternalInput").ap()
    # wmp[p, m, o] = Wm[o, (2m + p//64)*64 + p%64]: K chunk for head pair m
    wmp_d = nc.dram_tensor("wmp", [128, NC_CHUNK, C], BF16, kind="ExternalInput").ap()
    bq_d = nc.dram_tensor("bq", [128, NC_CHUNK], F32, kind="ExternalInput").ap()
    bk_d = nc.dram_tensor("bk", [128, NC_CHUNK], F32, kind="ExternalInput").ap()
    bm_d = nc.dram_tensor("bm", [128, NC_CHUNK], F32, kind="ExternalInput").ap()
    bv_d = nc.dram_tensor("bv", [1, C], F32, kind="ExternalInput").ap()
    # ind[h, o] = 1.0 if o // 64 == h else 0  (bcast inv-norm rows -> o lanes)
    ind_d = nc.dram_tensor("ind", [H, C], BF16, kind="ExternalInput").ap()
    # blk[o, h] = ind.T (accumulate per-head sums of squares)
    blk_d = nc.dram_tensor("blk", [C, H], BF16, kind="ExternalInput").ap()
    # colsel[p, h, m] = 1.0 if m == h (route head h's softmax sums to row h)
    colsel_d = nc.dram_tensor("colsel", [128, H, H], BF16, kind="ExternalInput").ap()
    # indp[k, m, p] = 1.0 if k == 2m + p//64 (bcast rec rows onto head pair m)
    indp_d = nc.dram_tensor("indp", [H, NC_CHUNK, 128], BF16, kind="ExternalInput").ap()
    outT_d = nc.dram_tensor("outT", [P * C, LQ], F32, kind="ExternalOutput").ap()

    with tile.TileContext(nc) as tc:
        with (
            tc.tile_pool(name="singles", bufs=1) as singles,
            tc.tile_pool(name="qin", bufs=2) as qin_pool,
            tc.tile_pool(name="kvin", bufs=2) as kvin_pool,
            tc.tile_pool(name="qh", bufs=2) as qh_pool,
            tc.tile_pool(name="kh", bufs=2) as kh_pool,
            tc.tile_pool(name="vh", bufs=2) as vh_pool,
            tc.tile_pool(name="sq", bufs=5) as sq_pool,
            tc.tile_pool(name="inv", bufs=2) as inv_pool,
            tc.tile_pool(name="nv", bufs=2) as nv_pool,
            tc.tile_pool(name="es", bufs=H) as es_pool,
            tc.tile_pool(name="rc", bufs=2) as rc_pool,
            tc.tile_pool(name="xT", bufs=8) as xT_pool,
            tc.tile_pool(name="outs", bufs=2) as out_pool,
            tc.tile_pool(name="ps", bufs=8, space="PSUM") as ps_pool,
        ):
            # ---- persistent tiles ----
            w_sb = {}
            for name, d in (("wq", wqT_d), ("wk", wkT_d), ("wv", wvT_d)):
                t = singles.tile([128, NC_CHUNK, C], BF16, tag=f"w_{name}")
                nc.sync.dma_start(out=t, in_=d.rearrange("(kc p) o -> p kc o", p=128))
                w_sb[name] = t
            wmp_sb = singles.tile([128, NC_CHUNK, C], BF16, tag="w_wm")
            nc.sync.dma_start(out=wmp_sb, in_=wmp_d)
            bq_sb = singles.tile([128, NC_CHUNK], F32, tag="bq")
            nc.sync.dma_start(out=bq_sb, in_=bq_d)
            bk_sb = singles.tile([128, NC_CHUNK], F32, tag="bk")
            nc.sync.dma_start(out=bk_sb, in_=bk_d)
            bm_sb = singles.tile([128, NC_CHUNK], F32, tag="bm")
            nc.sync.dma_start(out=bm_sb, in_=bm_d)
            bv_sb = singles.tile([128, C], F32, tag="bv")
            nc.sync.dma_start(out=bv_sb, in_=bv_d.to_broadcast([128, C]))
            ind_sb = singles.tile([H, C], BF16, tag="ind")
            nc.sync.dma_start(out=ind_sb, in_=ind_d)
            blk_sb = singles.tile([128, NC_CHUNK, H], BF16, tag="blk")
            nc.sync.dma_start(out=blk_sb, in_=blk_d.rearrange("(j p) h -> p j h", p=128))
            colsel_sb = singles.tile([128, H, H], BF16, tag="colsel")
            nc.sync.dma_start(out=colsel_sb, in_=colsel_d)
            indp_sb = singles.tile([H, NC_CHUNK, 128], BF16, tag="indp")
            nc.sync.dma_start(out=indp_sb, in_=indp_d)
            # bias rows for the bits-log exp seeds (s^-1/2 and s^-1)
            a_h, b_h = _bits_exp_coefs(-0.5)
            a_r, b_r = _bits_exp_coefs(-1.0)
            bh_sb = singles.tile([128, 1], F32, tag="bh")
            nc.vector.memset(bh_sb, b_h)
            br_sb = singles.tile([128, 1], F32, tag="br")
            nc.vector.memset(br_sb, b_r)
            # k-side seed bias folds the 1/sqrt(D) score scale into inv|k|
            bk8_sb = singles.tile([128, 1], F32, tag="bk8")
            nc.vector.memset(bk8_sb, b_h + float(np.log(0.125)))
            eps2_bits = int(np.float32(EPS2).view(np.int32))

            def rsqrt_rounds(out_r, s_ps, pool, npart, n, tag, p, nr, clamp,
                             bias_row=None):
                """out_r = s_ps ** p via bits-log exp seed + nr Newton steps.
                s_ps is a PSUM f32 AP; clamp applies max(s, eps^2) in the int
                domain (valid for positive floats). nr=0 writes the seed
                directly to out_r (bias_row can fold in a constant factor)."""
                a, b, bias = (a_h, b_h, bh_sb) if p == -0.5 else (a_r, b_r, br_sb)
                if bias_row is not None:
                    bias = bias_row
                bits_f = pool.tile([npart, n], F32, tag=tag + "b")
                if clamp:
                    nc.vector.tensor_scalar(
                        out=bits_f, in0=s_ps.bitcast(I32),
                        scalar1=eps2_bits, scalar2=None,
                        op0=mybir.AluOpType.max,
                    )
                else:
                    nc.vector.tensor_copy(bits_f, s_ps.bitcast(I32))
                if nr == 0:
                    nc.scalar.activation(
                        out=out_r, in_=bits_f, func=AF.Exp, scale=a,
                        bias=bias[0:npart, :],
                    )
                    return
                y = pool.tile([npart, n], F32, tag=tag + "y")
                nc.scalar.activation(
                    out=y, in_=bits_f, func=AF.Exp, scale=a, bias=bias[0:npart, :]
                )
                for it in range(nr):
                    t = pool.tile([npart, n], F32, tag=tag + "t")
                    if p == -0.5:
                        # t = s*y^2 ; y <- y*(1.5 - 0.5 t)
                        nc.vector.tensor_mul(t, y, y)
                        nc.vector.tensor_mul(t, t, s_ps)
                        nc.vector.tensor_scalar(
                            out=t, in0=t, scalar1=-0.5, scalar2=1.5,
                            op0=mybir.AluOpType.mult, op1=mybir.AluOpType.add,
                        )
                    else:
                        # t = s*y ; y <- y*(2 - t)
                        nc.vector.tensor_mul(t, y, s_ps)
                        nc.vector.tensor_scalar(
                            out=t, in0=t, scalar1=-1.0, scalar2=2.0,
                            op0=mybir.AluOpType.mult, op1=mybir.AluOpType.add,
                        )
                    last = it == nr - 1
                    yn = out_r if last else pool.tile([npart, n], F32, tag=tag + "y")
                    nc.vector.tensor_mul(yn, t, y)
                    y = yn

            def proj_mm(w, bias_sb, in_sb, n, hname):
                """Projection matmuls + bias + squared-norm accumulation.
                Emits all 16 projection matmuls first so the DVE bias/sq ops
                overlap later matmuls; norm matmuls go last.

                q: returns (h_sb, norm_ps [H, n]) for the classic broadcast
                   normalize.
                k: returns (h_sb, norm_ps [128, NC_CHUNK, H]) -- the norms
                   transposed to [lk, h] via 16 tiny-N matmuls, so 1/|k| can
                   ride the exp activation as a per-partition scale."""
                h_sb = (qh_pool if hname == "q" else kh_pool).tile(
                    [128, NC_CHUNK, n], BF16, tag=hname + "h"
                )
                ps_l = []
                for j in range(NC_CHUNK):
                    ps = ps_pool.tile([128, n], F32, tag="ps")
                    for kc in range(NC_CHUNK):
                        nc.tensor.matmul(
                            ps,
                            lhsT=w[:, kc, j * 128:(j + 1) * 128],
                            rhs=in_sb[:, kc, :],
                            start=(kc == 0),
                            stop=(kc == NC_CHUNK - 1),
                        )
                    ps_l.append(ps)
                sq_l = []
                for j in range(NC_CHUNK):
                    # add bias while copying PSUM -> SBUF
                    nc.vector.tensor_scalar(
                        out=h_sb[:, j, :], in0=ps_l[j],
                        scalar1=bias_sb[:, j:j + 1], scalar2=None,
                        op0=mybir.AluOpType.add,
                    )
                    sq = sq_pool.tile([128, n], BF16, tag="sq" + hname)
                    nc.vector.tensor_mul(sq, h_sb[:, j, :], h_sb[:, j, :])
                    sq_l.append(sq)
                if hname == "q":
                    norm_ps = ps_pool.tile([H, n], F32, tag="ps")
                    for j in range(NC_CHUNK):
                        nc.tensor.matmul(
                            norm_ps,
                            lhsT=blk_sb[:, j, :],
                            rhs=sq_l[j],
                            start=(j == 0),
                            stop=(j == NC_CHUNK - 1),
                        )
                else:
                    norm_ps = ps_pool.tile([128, NC_CHUNK, H], F32, tag="ps")
                    for jk in range(NC_CHUNK):      # lk chunk (output rows)
                        for j in range(NC_CHUNK):   # (h d) row chunk
                            nc.tensor.matmul(
                                norm_ps[:, jk, :],
                                lhsT=sq_l[j][:, jk * 128:(jk + 1) * 128],
                                rhs=blk_sb[:, j, :],
                                start=(j == 0),
                                stop=(j == NC_CHUNK - 1),
                            )
                return h_sb, norm_ps

            def apply_norm(h_sb, inv_sb, n):
                """h *= inv (broadcast head rows over their 64 partitions)."""
                for j in range(NC_CHUNK):
                    bc = ps_pool.tile([128, n], F32, tag="ps")
                    nc.tensor.matmul(
                        bc,
                        lhsT=ind_sb[:, j * 128:(j + 1) * 128],
                        rhs=inv_sb,
                        start=True, stop=True,
                    )
                    nc.vector.tensor_mul(h_sb[:, j, :], h_sb[:, j, :], bc)

            def emit_outproj(xT, q_sb, i):
                # ---- output projection + bias + residual (transposed) ----
                out_sb = out_pool.tile([128, NC_CHUNK, LQ], F32, tag="outs")
                for jo in range(NC_CHUNK):
                    ps_o = ps_pool.tile([128, LQ], F32, tag="ps")
                    for m in range(NC_CHUNK):  # K chunks of 128 (head pair)
                        nc.tensor.matmul(
                            ps_o,
                            lhsT=wmp_sb[:, m, jo * 128:(jo + 1) * 128],
                            rhs=xT[m],
                            start=(m == 0),
                            stop=(m == NC_CHUNK - 1),
                        )
                    # out = ps_o + bm + qT  (fused bias + residual)
                    nc.vector.scalar_tensor_tensor(
                        out=out_sb[:, jo, :],
                        in0=ps_o,
                        scalar=bm_sb[:, jo:jo + 1],
                        in1=q_sb[:, jo, :],
                        op0=mybir.AluOpType.add,
                        op1=mybir.AluOpType.add,
                    )
                nc.sync.dma_start(
                    out=outT_d[i * C:(i + 1) * C, :].rearrange(
                        "(j p) l -> p j l", p=128
                    ),
                    in_=out_sb,
                )

            prev = None
            for i in range(P):
                # ---- load inputs (transposed): q^T [c, lq], kv^T [c, lk] ----
                q_sb = qin_pool.tile([128, NC_CHUNK, LQ], BF16, tag="qin")
                nc.sync.dma_start(
                    out=q_sb,
                    in_=qT_d[i * C:(i + 1) * C, :].rearrange("(j p) l -> p j l", p=128),
                )
                kv_sb = kvin_pool.tile([128, NC_CHUNK, LK], BF16, tag="kvin")
                nc.sync.dma_start(
                    out=kv_sb,
                    in_=kvT_d[i * C:(i + 1) * C, :].rearrange("(j p) l -> p j l", p=128),
                )

                # ---- Q / K / V projection matmul blocks + inv-norm chains ----
                qh_sb, normq_ps = proj_mm(w_sb["wq"], bq_sb, q_sb, LQ, "q")
                invq_sb = inv_pool.tile([H, LQ], BF16, tag="invq")
                rsqrt_rounds(invq_sb, normq_ps, inv_pool, H, LQ, "invq",
                             p=-0.5, nr=0, clamp=True)

                kh_sb, normk_ps = proj_mm(w_sb["wk"], bk_sb, kv_sb, LK, "k")
                # invk_sc[lk, jk, h] = 0.125 / |k|: per-partition exp scale
                invk_sc = inv_pool.tile([128, NC_CHUNK, H], F32, tag="invk")
                rsqrt_rounds(invk_sc.rearrange("p a b -> p (a b)"),
                             normk_ps.rearrange("p a b -> p (a b)"),
                             inv_pool, 128, NC_CHUNK * H, "invk",
                             p=-0.5, nr=0, clamp=True, bias_row=bk8_sb)

                # V projection (natural layout): vh[lk, h, d] = kv @ Wv.T + bv
                vh_sb = vh_pool.tile([128, NC_CHUNK, H, D], BF16, tag="vh")
                nv_all = nv_pool.tile([128, NC_CHUNK, H], F32, tag="nv")
                ps_v = []
                for j in range(NC_CHUNK):  # lk chunk
                    ps = ps_pool.tile([128, C], F32, tag="ps")
                    for kc in range(NC_CHUNK):
                        nc.tensor.matmul(
                            ps,
                            lhsT=kv_sb[:, kc, j * 128:(j + 1) * 128],
                            rhs=w_sb["wv"][:, kc, :],
                            start=(kc == 0),
                            stop=(kc == NC_CHUNK - 1),
                        )
                    ps_v.append(ps)
                for j in range(NC_CHUNK):
                    nc.vector.tensor_add(
                        vh_sb[:, j, :, :],
                        ps_v[j].rearrange("p (h d) -> p h d", h=H),
                        bv_sb.rearrange("p (h d) -> p h d", h=H),
                    )
                    sqv = sq_pool.tile([128, C], BF16, tag="sqv")
                    sqv3 = sqv.rearrange("p (h d) -> p h d", h=H)
                    nc.vector.tensor_mul(
                        sqv3, vh_sb[:, j, :, :], vh_sb[:, j, :, :]
                    )
                    nc.vector.reduce_sum(
                        nv_all[:, j, :], sqv3, axis=mybir.AxisListType.X
                    )
                # rsqrt of all 32 v norms at once
                nvr = nv_pool.tile([128, NC_CHUNK, H], BF16, tag="nvr")
                rsqrt_rounds(
                    nvr.rearrange("p a b -> p (a b)"),
                    nv_all.rearrange("p a b -> p (a b)"),
                    nv_pool, 128, NC_CHUNK * H, "nv",
                    p=-0.5, nr=1, clamp=True,
                )

                # ---- output projection of the PREVIOUS pair: fills PE while
                # this pair's inverse-norm chains run on DVE/ACT ----
                if prev is not None:
                    emit_outproj(*prev)
                    prev = None

                # ---- apply the inverse norms (k rides the exp scale) ----
                apply_norm(qh_sb, invq_sb, LQ)
                for j in range(NC_CHUNK):
                    nc.vector.tensor_mul(
                        vh_sb[:, j, :, :],
                        vh_sb[:, j, :, :],
                        nvr[:, j, :].broadcast_to([128, H, D]),
                    )

                # ---- attention pass 1: scores + exp per head (es in SBUF) ----
                es_all = []
                for h in range(H):
                    jh, ph = h // 2, (h % 2) * D
                    es_sb = es_pool.tile([128, NC_CHUNK, LQ], BF16, tag="es")
                    for jkk in range(NC_CHUNK // 2):  # pairs of lk chunks
                        ps_s = ps_pool.tile([128, 2, LQ], F32, tag="ps")
                        for s in range(2):
                            jk = 2 * jkk + s
                            nc.tensor.matmul(
                                ps_s[:, s, :],
                                lhsT=kh_sb[ph:ph + D, jh, jk * 128:(jk + 1) * 128],
                                rhs=qh_sb[ph:ph + D, jh, :],
                                start=True, stop=True,
                            )
                        # att = exp(scores * 0.125/|k|); the inverse k norm
                        # rides along as a per-partition activation scale.
                        # Cosine scores are bounded so no max-subtraction.
                        for s in range(2):
                            jk = 2 * jkk + s
                            nc.scalar.activation(
                                out=es_sb[:, jk, :], in_=ps_s[:, s, :],
                                func=AF.Exp, scale=invk_sc[:, jk, h:h + 1],
                            )
                    es_all.append(es_sb)
                # all softmax denominators -> one [8, lq] PSUM tile (colsel
                # routes head h's column-sums to row h); emitted after ALL
                # score matmuls so PE never waits on a just-issued exp
                sums_ps = ps_pool.tile([H, LQ], F32, tag="ps")
                for h in range(H):
                    for jk in range(NC_CHUNK):
                        nc.tensor.matmul(
                            sums_ps,
                            lhsT=colsel_sb[:, h, :],
                            rhs=es_all[h][:, jk, :],
                            start=(h == 0 and jk == 0),
                            stop=(h == H - 1 and jk == NC_CHUNK - 1),
                        )
                # reciprocals of all 8 denominators: 1/s = s^-1
                rec_r = rc_pool.tile([H, LQ], BF16, tag="rcr")
                rsqrt_rounds(
                    rec_r, sums_ps, rc_pool, H, LQ, "rec", p=-1.0, nr=1, clamp=False
                )

                # ---- pass 2: PV + normalize, head pairs (2m, 2m+1) share a
                # [128, lq] PSUM tile -> x^T pair tiles with K=128 rows ----
                xT = []
                for m in range(NC_CHUNK):
                    ps_x = ps_pool.tile([128, LQ], F32, tag="ps")
                    for s in range(2):
                        h = 2 * m + s
                        for jk in range(NC_CHUNK):
                            nc.tensor.matmul(
                                ps_x[s * D:(s + 1) * D, :],
                                lhsT=vh_sb[:, jk, h, :],
                                rhs=es_all[h][:, jk, :],
                                start=(jk == 0),
                                stop=(jk == NC_CHUNK - 1),
                            )
                    bc = ps_pool.tile([128, LQ], F32, tag="ps")
                    nc.tensor.matmul(
                        bc, lhsT=indp_sb[:, m, :], rhs=rec_r,
                        start=True, stop=True,
                    )
                    # DVE cannot read two PSUM operands; stage bc via ACT
                    bc_sb = rc_pool.tile([128, LQ], F32, tag="bcsb")
                    nc.scalar.activation(out=bc_sb, in_=bc, func=AF.Identity)
                    xt = xT_pool.tile([128, LQ], BF16, tag="xT")
                    nc.vector.tensor_mul(xt, ps_x, bc_sb)
                    xT.append(xt)

                prev = (xT, q_sb, i)

            emit_outproj(*prev)

    nc.compile()
    return nc


_NC_CACHE = None


def _get_program():
    global _NC_CACHE
    if _NC_CACHE is None:
        _NC_CACHE = build_program()
    return _NC_CACHE


def prep_in_maps(q, kv, Wq, bq, Wkv, bkv, Wm, bm):
    import ml_dtypes

    bf16 = ml_dtypes.bfloat16
    q = np.ascontiguousarray(np.asarray(q, dtype=np.float32))
    kv = np.ascontiguousarray(np.asarray(kv, dtype=np.float32))
    b, t, lq, c = q.shape
    lk = kv.shape[2]
    npairs = b * t
    per_core = npairs // NCORES

    # host-side transposes / weight prep (not on the device critical path)
    qT = np.ascontiguousarray(
        q.reshape(npairs, lq, c).transpose(0, 2, 1).astype(bf16)
    )  # [64, c, lq]
    kvT = np.ascontiguousarray(
        kv.reshape(npairs, lk, c).transpose(0, 2, 1).astype(bf16)
    )  # [64, c, lk]
    wqT = np.ascontiguousarray(np.asarray(Wq, np.float32).T.astype(bf16))
    wkT = np.ascontiguousarray(np.asarray(Wkv[:C], np.float32).T.astype(bf16))
    wvT = np.ascontiguousarray(np.asarray(Wkv[C:], np.float32).T.astype(bf16))
    # wmp[p, m, o] = Wm[o, (2m + p//64)*64 + p%64]
    wmp = np.ascontiguousarray(
        np.asarray(Wm, np.float32).T.reshape(NC_CHUNK, 128, C)
        .transpose(1, 0, 2).astype(bf16)
    )
    bq_t = np.ascontiguousarray(np.asarray(bq, np.float32).reshape(NC_CHUNK, 128).T)
    bk_t = np.ascontiguousarray(
        np.asarray(bkv[:C], np.float32).reshape(NC_CHUNK, 128).T
    )
    bv_t = np.ascontiguousarray(np.asarray(bkv[C:], np.float32).reshape(1, C))
    bm_t = np.ascontiguousarray(np.asarray(bm, np.float32).reshape(NC_CHUNK, 128).T)
    ind = np.zeros((H, C), bf16)
    for h in range(H):
        ind[h, h * D:(h + 1) * D] = 1.0
    blk = np.ascontiguousarray(np.zeros((C, H), bf16))
    for h in range(H):
        blk[h * D:(h + 1) * D, h] = 1.0
    colsel = np.zeros((128, H, H), bf16)
    for h in range(H):
        colsel[:, h, h] = 1.0
    indp = np.zeros((H, NC_CHUNK, 128), bf16)
    for m in range(NC_CHUNK):
        indp[2 * m, m, 0:64] = 1.0
        indp[2 * m + 1, m, 64:128] = 1.0

    in_maps = []
    for core in range(NCORES):
        s = core * per_core
        e = s + per_core
        in_maps.append({
            "qT": qT[s:e].reshape(per_core * C, lq),
            "kvT": kvT[s:e].reshape(per_core * C, lk),
            "wqT": wqT, "wkT": wkT, "wvT": wvT, "wmp": wmp,
            "bq": bq_t, "bk": bk_t, "bv": bv_t, "bm": bm_t,
            "ind": ind, "blk": blk, "colsel": colsel, "indp": indp,
        })
    return in_maps, (b, t, lq, c, per_core)


def kernel(q, kv, Wq, bq, Wkv, bkv, Wm, bm):
    in_maps, (b, t, lq, c, per_core) = prep_in_maps(q, kv, Wq, bq, Wkv, bkv, Wm, bm)
    nc = _get_program()
    res = run_bass_kernel_spmd(nc, in_maps, core_ids=list(range(NCORES)))
    outT = np.concatenate(
        [res.results[core]["outT"].reshape(per_core, C, lq) for core in range(NCORES)],
        axis=0,
    )  # [64, c, lq]
    out = outT.transpose(0, 2, 1).reshape(b, t, lq, c)
    return np.ascontiguousarray(out)
